# revision 29
# baseline (speedup 1.0000x reference)
"""GaussianBasis rasterization on 8 Trainium2 NeuronCores (Bass/Tile).

Sharding: H*W pixel dim across 8 cores (32 rows each), per the hint.

Math: for pixel (x, y) and gaussian n,
  sigma = 0.5*c1*dx^2 + 0.5*c3*dy^2 + c2*dx*dy    (dx = cx-x, dy = cy-y)
is a low-rank form in chunk-local pixel monomials, so each 2-row x 256-col
pixel chunk's sigma tile is ONE K=12 fp32r matmul against per-(chunk,
gaussian) coefficients precomputed on the host (fp32r keeps 11 mantissa
bits; coefficients are hi/lo fp32r pairs and dx'^2 is an exact hi/lo row
pair, so each term carries ~2^-24 relative error).  alpha = exp(-sigma) on
ScalarE (the reference's alpha threshold + clamp change the output by rel
3.9e-3 << 2e-2 tolerance, and sigma >= 0 always since the conic is
positive definite, so they are skipped).  Output = feats.T @ alpha via
bf16 matmuls written to bf16 PSUM; stores are bf16 and upcast on host.

Gaussian support is tiny (|dy| <= sqrt(2*ln(255)*c) <= 6 rows), so each
4-row chunk pair only needs a window of the cy-sorted gaussian list (<= 69
for the reference inputs; capacity 96).  Windows are host-gathered into
dense per-core tensors so all 8 cores run one SPMD program; padding uses
zero features (exactly zero contribution).

A burst of dummy matmuls at program start (overlapping the input DMA
wait) warms the PE HAM clock gate from 1.2 to 2.4 GHz before real work.
"""

import numpy as np
import ml_dtypes

from concourse import bass, bacc, mybir
from concourse import tile
from concourse.bass_utils import run_bass_kernel_spmd

H = 256
W = 256
N_GAUSS = 1024
M_COMP = 50
NCH = 3 * M_COMP          # 150 output channels
NCHP = 160                # 128 + 32 (remainder padded to 32 for col tiling)
NCORES = 8
ROWS_PER_CORE = H // NCORES           # 32
CHUNK_ROWS = 2
NCHUNK = ROWS_PER_CORE // CHUNK_ROWS  # 16 sigma chunks
NPAIR = NCHUNK // 2                   # 8 feats pairs
PIX = CHUNK_ROWS * W                  # 512 pixels per sigma chunk
PIX2 = 2 * PIX                        # 1024 pixels per pair
KROWS = 12                # sigma matmul contraction rows (fp32r hi/lo pairs)
N_PRIMER = 10             # PE warm-up matmuls at start
LOG255 = float(np.log(255.0))

_cache = {}


def _to_f32r(a):
    """Round to the fp32r grid: fp32 with the low 12 mantissa bits dropped
    (round-to-nearest-even), matching walrus's fp32_to_fp32r."""
    f = np.asarray(a, np.float64).astype(np.float32)
    u = f.view(np.uint32)
    low = u & np.uint32(0xFFF)
    base = u & ~np.uint32(0xFFF)
    tie_up = (low > 0x800) | ((low == 0x800) & (((u >> 12) & 1) == 1))
    r = base + np.where(tie_up, np.uint32(0x1000), np.uint32(0))
    return r.view(np.float32)


def _build_nc(cap):
    f32 = mybir.dt.float32
    f32r = mybir.dt.float32r
    bf16 = mybir.dt.bfloat16
    nc = bacc.Bacc(None, target_bir_lowering=False)
    # Row/col group placement: the two sigma matmuls of a pair contract on
    # PE rows 64-75 / 96-107 and write PE cols 0-63 / 64-127; the feats
    # matmuls of chunk h contract on rows 64*h..64*h+63 (featsw and the
    # exp'd weights are duplicated across both row halves).  Each matmul
    # in a pair is row-disjoint from its neighbor, so consecutive matmuls
    # overlap in the array (measured ~3x for 4-way row tiling).
    gmono_d = nc.declare_dram_parameter("gmono", [KROWS, PIX + NCHUNK * cap],
                                        f32r, isOutput=False)
    featsw_d = nc.declare_dram_parameter("featsw", [cap, NPAIR * NCHP], bf16,
                                         isOutput=False)
    out_d = nc.declare_dram_parameter("out", [NCH, ROWS_PER_CORE * W], bf16,
                                      isOutput=True)
    EXP = mybir.ActivationFunctionType.Exp

    with tile.TileContext(nc) as tc:
        with tc.tile_pool(name="const", bufs=1) as constp, \
             tc.tile_pool(name="wgt", bufs=3) as wp, \
             tc.tile_pool(name="outs", bufs=2) as op_, \
             tc.tile_pool(name="ps", bufs=2, space=bass.MemorySpace.PSUM) as pp:
            gmono = constp.tile([96 + KROWS, PIX + NCHUNK * cap], f32r)
            nc.sync.dma_start(out=gmono[64:64 + KROWS, :], in_=gmono_d[:])
            nc.sync.dma_start(out=gmono[96:96 + KROWS, :], in_=gmono_d[:])
            featsw = constp.tile([128, NPAIR * NCHP], bf16)
            nc.scalar.dma_start(out=featsw[0:cap, :], in_=featsw_d[:])
            nc.scalar.dma_start(out=featsw[64:64 + cap, :], in_=featsw_d[:])

            wgs = {}

            def sigma_pair(k):
                """Both sigma chunks of pair k in concurrent PE row groups
                (rows 64-75 / 96-107) into one 2-bank PSUM tile, one exp."""
                sg = pp.tile([cap, PIX2], f32, tag="sig", bufs=1,
                             name=f"sg{k}")
                for h in range(2):
                    gcol = PIX + (2 * k + h) * cap
                    rb = 64 + 32 * h
                    nc.tensor.matmul(sg[:, h * PIX:(h + 1) * PIX],
                                     gmono[rb:rb + KROWS, gcol:gcol + cap],
                                     gmono[rb:rb + KROWS, 0:PIX],
                                     tile_position=(rb, 0),
                                     start=True, stop=True)
                wg = wp.tile([128, PIX2], bf16, tag="w", name=f"wg{k}")
                nc.scalar.activation(wg[0:cap, :], sg[:], EXP, scale=-1.0)
                nc.gpsimd.dma_start(out=wg[64:64 + cap, :], in_=wg[0:cap, :])
                wgs[k] = wg

            sigma_pair(0)
            sigma_pair(1)
            for k in range(NPAIR):               # pair loop
                if k % 2 == 0:
                    oas = op_.tile([128, 2 * PIX2], bf16, tag="oas")
                obp = pp.tile([128, PIX], f32, tag="obp", bufs=2,
                              name=f"obp{k}")
                obs = op_.tile([64, PIX], bf16, tag="obs", bufs=2,
                               name=f"obs{k}")
                wg = wgs.pop(k)
                oa = pp.tile([128, PIX2], f32, tag="oa", bufs=2,
                             name=f"oa{k}")
                for h in range(2):               # concurrent row halves
                    rb = 64 * h
                    px = slice(h * PIX, (h + 1) * PIX)
                    nc.tensor.matmul(oa[:, px],
                                     featsw[rb:rb + cap,
                                            k * NCHP:k * NCHP + 128],
                                     wg[rb:rb + cap, px],
                                     start=True, stop=True)
                for h in range(2):               # concurrent row halves
                    rb = 64 * h
                    px = slice(h * PIX, (h + 1) * PIX)
                    # both chunks' remainder channels in ONE psum bank:
                    # chunk h at partitions 64+32h..95+32h
                    nc.tensor.matmul(
                        obp[64 + 32 * h:96 + 32 * h, 0:PIX],
                        featsw[rb:rb + cap,
                               k * NCHP + 128:(k + 1) * NCHP],
                        wg[rb:rb + cap, px], tile_position=(rb, 64 + 32 * h),
                        start=True, stop=True)
                if k + 2 < NPAIR:                # PE runs sigma 2 pairs ahead
                    sigma_pair(k + 2)
                nc.vector.tensor_copy(oas[:, (k % 2) * PIX2:(k % 2 + 1) * PIX2],
                                      oa[:])
                nc.sync.dma_start(
                    out=out_d[0:128, k * PIX2:(k + 1) * PIX2],
                    in_=oas[:, (k % 2) * PIX2:(k % 2 + 1) * PIX2])
                nc.vector.tensor_copy(obs[:], obp[64:128, :])
                for h in range(2):
                    nc.sync.dma_start(
                        out=out_d[128:NCH, (2 * k + h) * PIX:
                                  (2 * k + h + 1) * PIX],
                        in_=obs[32 * h:32 * h + 22, :])
    nc.compile()
    return nc


def _host_precompute(xyz_raw, cholesky_raw, opacity, features_dc, cluster_id):
    """Returns (cap, pmono, per-core gmat list, per-core featsw list)."""
    xyz = np.asarray(xyz_raw, np.float64)
    chol = np.asarray(cholesky_raw, np.float64)
    feats = np.asarray(features_dc, np.float64)[int(cluster_id)]  # [M, N, 3]

    xy = np.tanh(xyz)
    c = chol + np.array([0.5, 0.0, 0.5])
    l1, l2, l3 = c[:, 0], c[:, 1], c[:, 2]
    a = l1 * l1
    b = l1 * l2
    cc = l2 * l2 + l3 * l3
    det = a * cc - b * b
    c1, c2, c3 = cc / det, -b / det, a / det
    cx = 0.5 * ((xy[:, 0] + 1.0) * W - 1.0)
    cy = 0.5 * ((xy[:, 1] + 1.0) * H - 1.0)
    # opacity folds into the constant coefficient:
    # alpha = op*exp(-sigma) = exp(-(sigma - ln(op)))
    op = np.asarray(opacity, np.float64)[:, 0]
    ry = np.sqrt(np.maximum(2.0 * (LOG255 + np.log(np.maximum(op, 1e-30))),
                            0.0) * cc)

    # exact relevant-gaussian sets per 4-row pair (margin 0.25 for float
    # safety); max count measured 56 <= 64
    n_pairs_all = H // (2 * CHUNK_ROWS)
    psets = []
    for k in range(n_pairs_all):
        r0 = k * 2 * CHUNK_ROWS
        r1 = r0 + 2 * CHUNK_ROWS - 1
        rel = (cy + ry >= r0 - 0.25) & (cy - ry <= r1 + 0.25)
        psets.append(np.nonzero(rel)[0])
    maxcnt = max(len(s) for s in psets)
    cap = 64
    while cap < maxcnt:
        cap += 32
    assert cap <= 64, f"pair gaussian count {maxcnt} exceeds row-split cap"

    # feats_r[n, m*3+ch] = feats[m, n, ch]
    feats_r = np.ascontiguousarray(
        feats.transpose(1, 0, 2).reshape(N_GAUSS, NCH)).astype(np.float32)

    # fp32r-exact local monomial rows (dx' in [-127.5, 127.5] half-integers,
    # dy' = +-0.5); dx'^2 split into an exact hi/lo fp32r row pair.
    jj = np.arange(PIX)
    dxl = (jj % W) - 127.5
    dyl = (jj // W) - 0.5
    dx2 = dxl * dxl
    dx2_hi = _to_f32r(dx2).astype(np.float64)
    dx2_lo = dx2 - dx2_hi
    pmono = np.stack([dx2_hi, dx2_hi, dx2_lo, dyl * dyl,
                      dxl * dyl, dxl * dyl, dxl, dxl, dyl, dyl,
                      np.ones(PIX), np.ones(PIX)])

    gmats = []
    featsws = []
    for core in range(NCORES):
        gm = np.zeros((KROWS, NCHUNK * cap), np.float64)
        fw = np.zeros((cap, NPAIR * NCHP), np.float32)
        for q in range(NPAIR):
            k = core * NPAIR + q
            g = psets[k]
            cnt = len(g)
            if cnt == 0:
                continue
            fw[:cnt, q * NCHP:q * NCHP + NCH] = feats_r[g]
            gx = cx[g] - 127.5
            for h in range(2):
                pch = 2 * q + h
                gy = cy[g] - ((core * NCHUNK + pch) * CHUNK_ROWS + 0.5)
                # sigma = D*dx'^2 + E*dy'^2 + F*dx'dy' + B*dx' + C*dy' + A
                col = slice(pch * cap, pch * cap + cnt)
                D = 0.5 * c1[g]
                E = 0.5 * c3[g]
                F = c2[g]
                B = -(c1[g] * gx + c2[g] * gy)
                C = -(c3[g] * gy + c2[g] * gx)
                A = (0.5 * c1[g] * gx * gx + 0.5 * c3[g] * gy * gy
                     + c2[g] * gx * gy - np.log(np.maximum(op[g], 1e-30)))
                Dh = _to_f32r(D)
                Fh = _to_f32r(F)
                Bh = _to_f32r(B)
                Ch = _to_f32r(C)
                Ah = _to_f32r(A)
                gm[0, col] = Dh                  # * dx2_hi
                gm[1, col] = D - Dh             # * dx2_hi
                gm[2, col] = D                  # * dx2_lo
                gm[3, col] = E                  # * dy'^2
                gm[4, col] = Fh                 # * dx'dy'
                gm[5, col] = F - Fh
                gm[6, col] = Bh                 # * dx'
                gm[7, col] = B - Bh
                gm[8, col] = Ch                 # * dy'
                gm[9, col] = C - Ch
                gm[10, col] = Ah                # * 1
                gm[11, col] = A - Ah
        gmats.append(_to_f32r(gm))
        featsws.append(fw.astype(ml_dtypes.bfloat16))
    return cap, _to_f32r(pmono), gmats, featsws


def _in_maps(xyz_raw, cholesky_raw, opacity, features_dc, cluster_id):
    cap, pmono, gmats, featsws = _host_precompute(
        xyz_raw, cholesky_raw, opacity, features_dc, cluster_id)
    in_maps = [{"gmono": np.concatenate([pmono, gmats[c]], axis=1),
                "featsw": featsws[c]}
               for c in range(NCORES)]
    return cap, in_maps


def _assemble(results):
    full = np.concatenate([np.asarray(r["out"], np.float32)
                           for r in results], axis=1)      # [150, H*W]
    return np.ascontiguousarray(full.reshape(M_COMP, 3, H, W))


def _get_nc(cap):
    if cap not in _cache:
        _cache[cap] = _build_nc(cap)
    return _cache[cap]


def kernel(xyz_raw, cholesky_raw, opacity, features_dc, cluster_id):
    cap, in_maps = _in_maps(xyz_raw, cholesky_raw, opacity, features_dc,
                            cluster_id)
    nc = _get_nc(cap)
    res = run_bass_kernel_spmd(nc, in_maps, list(range(NCORES)))
    return _assemble(res.results)


def kernel_traced(xyz_raw, cholesky_raw, opacity, features_dc, cluster_id,
                  **trace_kwargs):
    """For test.py: returns (output, BassKernelResults with profile)."""
    cap, in_maps = _in_maps(xyz_raw, cholesky_raw, opacity, features_dc,
                            cluster_id)
    nc = _get_nc(cap)
    res = run_bass_kernel_spmd(nc, in_maps, list(range(NCORES)), trace=True,
                               **trace_kwargs)
    return _assemble(res.results), res


# revision 30
# speedup vs baseline: 1.0801x; 1.0801x over previous
"""GaussianBasis rasterization on 8 Trainium2 NeuronCores (Bass/Tile).

Sharding: H*W pixel dim across 8 cores (32 rows each), per the hint.

Math: for pixel (x, y) and gaussian n,
  sigma = 0.5*c1*dx^2 + 0.5*c3*dy^2 + c2*dx*dy    (dx = cx-x, dy = cy-y)
is a low-rank form in chunk-local pixel monomials, so each 2-row x 256-col
pixel chunk's sigma tile is ONE K=12 fp32r matmul against per-(chunk,
gaussian) coefficients precomputed on the host (fp32r keeps 11 mantissa
bits; coefficients are hi/lo fp32r pairs and dx'^2 is an exact hi/lo row
pair, so each term carries ~2^-24 relative error).  alpha = exp(-sigma) on
ScalarE (the reference's alpha threshold + clamp change the output by rel
3.9e-3 << 2e-2 tolerance, and sigma >= 0 always since the conic is
positive definite, so they are skipped).  Output = feats.T @ alpha via
bf16 matmuls written to bf16 PSUM; stores are bf16 and upcast on host.

Gaussian support is tiny (|dy| <= sqrt(2*ln(255)*c) <= 6 rows), so each
4-row chunk pair only needs a window of the cy-sorted gaussian list (<= 69
for the reference inputs; capacity 96).  Windows are host-gathered into
dense per-core tensors so all 8 cores run one SPMD program; padding uses
zero features (exactly zero contribution).

A burst of dummy matmuls at program start (overlapping the input DMA
wait) warms the PE HAM clock gate from 1.2 to 2.4 GHz before real work.
"""

import numpy as np
import ml_dtypes

from concourse import bass, bacc, mybir
from concourse import tile
from concourse.bass_utils import run_bass_kernel_spmd

H = 256
W = 256
N_GAUSS = 1024
M_COMP = 50
NCH = 3 * M_COMP          # 150 output channels
NCHP = 160                # 128 + 32 (remainder padded to 32 for col tiling)
NCORES = 8
ROWS_PER_CORE = H // NCORES           # 32
CHUNK_ROWS = 2
NCHUNK = ROWS_PER_CORE // CHUNK_ROWS  # 16 sigma chunks
NPAIR = NCHUNK // 2                   # 8 feats pairs
PIX = CHUNK_ROWS * W                  # 512 pixels per sigma chunk
PIX2 = 2 * PIX                        # 1024 pixels per pair
KROWS = 12                # sigma matmul contraction rows (fp32r hi/lo pairs)
N_PRIMER = 10             # PE warm-up matmuls at start
LOG255 = float(np.log(255.0))

_cache = {}


def _to_f32r(a):
    """Round to the fp32r grid: fp32 with the low 12 mantissa bits dropped
    (round-to-nearest-even), matching walrus's fp32_to_fp32r."""
    f = np.asarray(a, np.float64).astype(np.float32)
    u = f.view(np.uint32)
    low = u & np.uint32(0xFFF)
    base = u & ~np.uint32(0xFFF)
    tie_up = (low > 0x800) | ((low == 0x800) & (((u >> 12) & 1) == 1))
    r = base + np.where(tie_up, np.uint32(0x1000), np.uint32(0))
    return r.view(np.float32)


def _build_nc(cap):
    f32 = mybir.dt.float32
    f32r = mybir.dt.float32r
    bf16 = mybir.dt.bfloat16
    nc = bacc.Bacc(None, target_bir_lowering=False)
    # Row/col group placement: the two sigma matmuls of a pair contract on
    # PE rows 64-75 / 96-107 and write PE cols 0-63 / 64-127; the feats
    # matmuls of chunk h contract on rows 64*h..64*h+63 (featsw and the
    # exp'd weights are duplicated across both row halves).  Each matmul
    # in a pair is row-disjoint from its neighbor, so consecutive matmuls
    # overlap in the array (measured ~3x for 4-way row tiling).
    gmono_d = nc.declare_dram_parameter("gmono", [KROWS, PIX + NCHUNK * cap],
                                        f32r, isOutput=False)
    featsw_d = nc.declare_dram_parameter("featsw", [cap, NPAIR * NCHP], bf16,
                                         isOutput=False)
    out_d = nc.declare_dram_parameter("out", [NCH, ROWS_PER_CORE * W], bf16,
                                      isOutput=True)
    EXP = mybir.ActivationFunctionType.Exp

    with tile.TileContext(nc) as tc:
        with tc.tile_pool(name="const", bufs=1) as constp, \
             tc.tile_pool(name="wgt", bufs=4) as wp, \
             tc.tile_pool(name="outs", bufs=2) as op_, \
             tc.tile_pool(name="ps", bufs=2, space=bass.MemorySpace.PSUM) as pp:
            gmono = constp.tile([96 + KROWS, PIX + NCHUNK * cap], f32r)
            HEAD = PIX + 4 * cap
            nc.sync.dma_start(out=gmono[64:64 + KROWS, 0:HEAD],
                              in_=gmono_d[:, 0:HEAD])
            nc.sync.dma_start(out=gmono[96:96 + KROWS, 0:HEAD],
                              in_=gmono_d[:, 0:HEAD])
            nc.sync.dma_start(out=gmono[64:64 + KROWS, HEAD:],
                              in_=gmono_d[:, HEAD:])
            nc.sync.dma_start(out=gmono[96:96 + KROWS, HEAD:],
                              in_=gmono_d[:, HEAD:])
            featsw = constp.tile([cap, NPAIR * NCHP], bf16)
            nc.scalar.dma_start(out=featsw[:], in_=featsw_d[:])

            wgs = {}

            def sigma_pair(k):
                """Both sigma chunks of pair k in concurrent PE row groups
                (rows 64-75 / 96-107) into one 2-bank PSUM tile, one exp."""
                sg = pp.tile([cap, PIX2], f32, tag="sig", bufs=1,
                             name=f"sg{k}")
                for h in range(2):
                    gcol = PIX + (2 * k + h) * cap
                    rb = 64 + 32 * h
                    nc.tensor.matmul(sg[:, h * PIX:(h + 1) * PIX],
                                     gmono[rb:rb + KROWS, gcol:gcol + cap],
                                     gmono[rb:rb + KROWS, 0:PIX],
                                     tile_position=(rb, 0),
                                     start=True, stop=True)
                wg = wp.tile([cap, PIX2], bf16, tag="w", name=f"wg{k}")
                nc.scalar.activation(wg[:], sg[:], EXP, scale=-1.0)
                wgs[k] = wg

            sigma_pair(0)
            sigma_pair(1)
            for k in range(NPAIR):               # pair loop
                if k % 2 == 0:
                    oas = op_.tile([128, 2 * PIX2], bf16, tag="oas")
                    obp = pp.tile([128, PIX2], f32, tag="obp", bufs=1)
                    obs = op_.tile([64, PIX2], bf16, tag="obs")
                wg = wgs.pop(k)
                oa = pp.tile([128, PIX2], f32, tag="oa", bufs=2,
                             name=f"oa{k}")
                cslot = 64 + 32 * (k % 2)
                for h in range(2):
                    px = slice(h * PIX, (h + 1) * PIX)
                    nc.tensor.matmul(oa[:, px],
                                     featsw[:, k * NCHP:k * NCHP + 128],
                                     wg[:, px], start=True, stop=True)
                    nc.tensor.matmul(
                        obp[cslot:cslot + 32, px],
                        featsw[:, k * NCHP + 128:(k + 1) * NCHP],
                        wg[:, px], tile_position=(0, cslot),
                        start=True, stop=True)
                if k + 2 < NPAIR:                # PE runs sigma 2 pairs ahead
                    sigma_pair(k + 2)
                nc.vector.tensor_copy(oas[:, (k % 2) * PIX2:(k % 2 + 1) * PIX2],
                                      oa[:])
                nc.sync.dma_start(
                    out=out_d[0:128, k * PIX2:(k + 1) * PIX2],
                    in_=oas[:, (k % 2) * PIX2:(k % 2 + 1) * PIX2])
                if k % 2 == 1:
                    nc.vector.tensor_copy(obs[:], obp[64:128, :])
                    g2 = k // 2
                    for kk in range(2):          # the 2 pairs in this group
                        kp = g2 * 2 + kk
                        nc.sync.dma_start(
                            out=out_d[128:NCH, kp * PIX2:(kp + 1) * PIX2],
                            in_=obs[32 * kk:32 * kk + 22, :])
    nc.compile()
    return nc


def _host_precompute(xyz_raw, cholesky_raw, opacity, features_dc, cluster_id):
    """Returns (cap, pmono, per-core gmat list, per-core featsw list)."""
    xyz = np.asarray(xyz_raw, np.float64)
    chol = np.asarray(cholesky_raw, np.float64)
    feats = np.asarray(features_dc, np.float64)[int(cluster_id)]  # [M, N, 3]

    xy = np.tanh(xyz)
    c = chol + np.array([0.5, 0.0, 0.5])
    l1, l2, l3 = c[:, 0], c[:, 1], c[:, 2]
    a = l1 * l1
    b = l1 * l2
    cc = l2 * l2 + l3 * l3
    det = a * cc - b * b
    c1, c2, c3 = cc / det, -b / det, a / det
    cx = 0.5 * ((xy[:, 0] + 1.0) * W - 1.0)
    cy = 0.5 * ((xy[:, 1] + 1.0) * H - 1.0)
    # opacity folds into the constant coefficient:
    # alpha = op*exp(-sigma) = exp(-(sigma - ln(op)))
    op = np.asarray(opacity, np.float64)[:, 0]
    ry = np.sqrt(np.maximum(2.0 * (LOG255 + np.log(np.maximum(op, 1e-30))),
                            0.0) * cc)

    # exact relevant-gaussian sets per 4-row pair (margin 0.25 for float
    # safety); max count measured 56 <= 64
    n_pairs_all = H // (2 * CHUNK_ROWS)
    psets = []
    for k in range(n_pairs_all):
        r0 = k * 2 * CHUNK_ROWS
        r1 = r0 + 2 * CHUNK_ROWS - 1
        rel = (cy + ry >= r0 - 0.25) & (cy - ry <= r1 + 0.25)
        psets.append(np.nonzero(rel)[0])
    maxcnt = max(len(s) for s in psets)
    cap = 64
    while cap < maxcnt:
        cap += 32
    assert cap <= 64, f"pair gaussian count {maxcnt} exceeds row-split cap"

    # feats_r[n, m*3+ch] = feats[m, n, ch]
    feats_r = np.ascontiguousarray(
        feats.transpose(1, 0, 2).reshape(N_GAUSS, NCH)).astype(np.float32)

    # fp32r-exact local monomial rows (dx' in [-127.5, 127.5] half-integers,
    # dy' = +-0.5); dx'^2 split into an exact hi/lo fp32r row pair.
    jj = np.arange(PIX)
    dxl = (jj % W) - 127.5
    dyl = (jj // W) - 0.5
    dx2 = dxl * dxl
    dx2_hi = _to_f32r(dx2).astype(np.float64)
    dx2_lo = dx2 - dx2_hi
    pmono = np.stack([dx2_hi, dx2_hi, dx2_lo, dyl * dyl,
                      dxl * dyl, dxl * dyl, dxl, dxl, dyl, dyl,
                      np.ones(PIX), np.ones(PIX)])

    gmats = []
    featsws = []
    for core in range(NCORES):
        gm = np.zeros((KROWS, NCHUNK * cap), np.float64)
        fw = np.zeros((cap, NPAIR * NCHP), np.float32)
        for q in range(NPAIR):
            k = core * NPAIR + q
            g = psets[k]
            cnt = len(g)
            if cnt == 0:
                continue
            fw[:cnt, q * NCHP:q * NCHP + NCH] = feats_r[g]
            gx = cx[g] - 127.5
            for h in range(2):
                pch = 2 * q + h
                gy = cy[g] - ((core * NCHUNK + pch) * CHUNK_ROWS + 0.5)
                # sigma = D*dx'^2 + E*dy'^2 + F*dx'dy' + B*dx' + C*dy' + A
                col = slice(pch * cap, pch * cap + cnt)
                D = 0.5 * c1[g]
                E = 0.5 * c3[g]
                F = c2[g]
                B = -(c1[g] * gx + c2[g] * gy)
                C = -(c3[g] * gy + c2[g] * gx)
                A = (0.5 * c1[g] * gx * gx + 0.5 * c3[g] * gy * gy
                     + c2[g] * gx * gy - np.log(np.maximum(op[g], 1e-30)))
                Dh = _to_f32r(D)
                Fh = _to_f32r(F)
                Bh = _to_f32r(B)
                Ch = _to_f32r(C)
                Ah = _to_f32r(A)
                gm[0, col] = Dh                  # * dx2_hi
                gm[1, col] = D - Dh             # * dx2_hi
                gm[2, col] = D                  # * dx2_lo
                gm[3, col] = E                  # * dy'^2
                gm[4, col] = Fh                 # * dx'dy'
                gm[5, col] = F - Fh
                gm[6, col] = Bh                 # * dx'
                gm[7, col] = B - Bh
                gm[8, col] = Ch                 # * dy'
                gm[9, col] = C - Ch
                gm[10, col] = Ah                # * 1
                gm[11, col] = A - Ah
        gmats.append(_to_f32r(gm))
        featsws.append(fw.astype(ml_dtypes.bfloat16))
    return cap, _to_f32r(pmono), gmats, featsws


def _in_maps(xyz_raw, cholesky_raw, opacity, features_dc, cluster_id):
    cap, pmono, gmats, featsws = _host_precompute(
        xyz_raw, cholesky_raw, opacity, features_dc, cluster_id)
    in_maps = [{"gmono": np.concatenate([pmono, gmats[c]], axis=1),
                "featsw": featsws[c]}
               for c in range(NCORES)]
    return cap, in_maps


def _assemble(results):
    full = np.concatenate([np.asarray(r["out"], np.float32)
                           for r in results], axis=1)      # [150, H*W]
    return np.ascontiguousarray(full.reshape(M_COMP, 3, H, W))


def _get_nc(cap):
    if cap not in _cache:
        _cache[cap] = _build_nc(cap)
    return _cache[cap]


def kernel(xyz_raw, cholesky_raw, opacity, features_dc, cluster_id):
    cap, in_maps = _in_maps(xyz_raw, cholesky_raw, opacity, features_dc,
                            cluster_id)
    nc = _get_nc(cap)
    res = run_bass_kernel_spmd(nc, in_maps, list(range(NCORES)))
    return _assemble(res.results)


def kernel_traced(xyz_raw, cholesky_raw, opacity, features_dc, cluster_id,
                  **trace_kwargs):
    """For test.py: returns (output, BassKernelResults with profile)."""
    cap, in_maps = _in_maps(xyz_raw, cholesky_raw, opacity, features_dc,
                            cluster_id)
    nc = _get_nc(cap)
    res = run_bass_kernel_spmd(nc, in_maps, list(range(NCORES)), trace=True,
                               **trace_kwargs)
    return _assemble(res.results), res


# revision 34
# speedup vs baseline: 1.0805x; 1.0004x over previous
"""GaussianBasis rasterization on 8 Trainium2 NeuronCores (Bass/Tile).

Sharding: the H*W pixel dim across 8 cores (32 rows each), per the hint.

Math: for pixel (x, y) and gaussian n,
  sigma = 0.5*c1*dx^2 + 0.5*c3*dy^2 + c2*dx*dy    (dx = cx-x, dy = cy-y)
is a low-rank form in chunk-local pixel monomials, so each 2-row x 256-col
pixel chunk's sigma tile is ONE K=12 fp32r matmul against per-(chunk,
gaussian) coefficients precomputed on the host (fp32r keeps 11 mantissa
bits; coefficients are hi/lo fp32r pairs and dx'^2 is an exact hi/lo row
pair, so each sigma term carries ~2^-24 relative error).  alpha =
exp(-sigma) on ScalarE (the reference's alpha threshold and clamp change
the output by rel 3.9e-3 << 2e-2 tolerance, and sigma >= 0 always since
the conic is positive definite, so both are skipped).  Output =
feats.T @ alpha via bf16 matmuls; stores are bf16 and upcast on host.

Gaussian support is tiny (|dy| <= sqrt(2*ln(255)*c) <= 6 rows), so each
4-row chunk pair only needs its exact relevant-gaussian set (<= 56 for the
reference inputs; capacity 64).  Sets are host-gathered into dense
per-core tensors so all 8 cores run one SPMD program; padding uses zero
features (exactly zero contribution) and zero sigma coefficients.

PE layout: feats matmuls contract on rows 0-63; the two sigma matmuls of
a pair contract on rows 64-75 and 96-107 (disjoint row groups), so they
run concurrently with each other and their weight loads pull ahead under
in-flight feats matmuls.  The 22 remainder output channels are padded to
32 and col-tiled into one shared PSUM tile per 2-pair group.  Outputs
stream out per pair as bf16 (halving HBM write traffic).
"""

import numpy as np
import ml_dtypes

from concourse import bass, bacc, mybir
from concourse import tile
from concourse.bass_utils import run_bass_kernel_spmd

H = 256
W = 256
N_GAUSS = 1024
M_COMP = 50
NCH = 3 * M_COMP          # 150 output channels
NCHP = 160                # 128 + 32 (remainder padded to 32 for col tiling)
NCORES = 8
ROWS_PER_CORE = H // NCORES           # 32
CHUNK_ROWS = 2
NCHUNK = ROWS_PER_CORE // CHUNK_ROWS  # 16 sigma chunks
NPAIR = NCHUNK // 2                   # 8 feats pairs
PIX = CHUNK_ROWS * W                  # 512 pixels per sigma chunk
PIX2 = 2 * PIX                        # 1024 pixels per pair
KROWS = 12                # sigma matmul contraction rows (fp32r hi/lo pairs)
N_PRIMER = 10             # PE warm-up matmuls at start
LOG255 = float(np.log(255.0))

_cache = {}


def _to_f32r(a):
    """Round to the fp32r grid: fp32 with the low 12 mantissa bits dropped
    (round-to-nearest-even), matching walrus's fp32_to_fp32r."""
    f = np.asarray(a, np.float64).astype(np.float32)
    u = f.view(np.uint32)
    low = u & np.uint32(0xFFF)
    base = u & ~np.uint32(0xFFF)
    tie_up = (low > 0x800) | ((low == 0x800) & (((u >> 12) & 1) == 1))
    r = base + np.where(tie_up, np.uint32(0x1000), np.uint32(0))
    return r.view(np.float32)


def _build_nc(cap):
    f32 = mybir.dt.float32
    f32r = mybir.dt.float32r
    bf16 = mybir.dt.bfloat16
    nc = bacc.Bacc(None, target_bir_lowering=False)
    # Row/col group placement: the two sigma matmuls of a pair contract on
    # PE rows 64-75 / 96-107 and write PE cols 0-63 / 64-127; the feats
    # matmuls of chunk h contract on rows 64*h..64*h+63 (featsw and the
    # exp'd weights are duplicated across both row halves).  Each matmul
    # in a pair is row-disjoint from its neighbor, so consecutive matmuls
    # overlap in the array (measured ~3x for 4-way row tiling).
    gmono_d = nc.declare_dram_parameter("gmono", [KROWS, PIX + NCHUNK * cap],
                                        f32r, isOutput=False)
    featsw_d = nc.declare_dram_parameter("featsw", [cap, NPAIR * NCHP], bf16,
                                         isOutput=False)
    out_d = nc.declare_dram_parameter("out", [NCH, ROWS_PER_CORE * W], bf16,
                                      isOutput=True)
    EXP = mybir.ActivationFunctionType.Exp

    with tile.TileContext(nc) as tc:
        with tc.tile_pool(name="const", bufs=1) as constp, \
             tc.tile_pool(name="wgt", bufs=4) as wp, \
             tc.tile_pool(name="outs", bufs=2) as op_, \
             tc.tile_pool(name="ps", bufs=2, space=bass.MemorySpace.PSUM) as pp:
            # sigma contraction-row placement: with cap <= 64 the feats
            # matmuls use PE rows 0-63, leaving row groups 2/3 for two
            # concurrent sigma streams; bigger caps fall back to serial
            # sigma in whatever rows remain free (correct, slower).
            if cap <= 64:
                rbs = (64, 96)
            elif cap <= 96:
                rbs = (96, 96)
            else:
                rbs = (0, 0)
            gmono = constp.tile([max(rbs) + KROWS, PIX + NCHUNK * cap], f32r)
            HEAD = PIX + 4 * cap
            for rb in sorted(set(rbs)):
                nc.sync.dma_start(out=gmono[rb:rb + KROWS, 0:HEAD],
                                  in_=gmono_d[:, 0:HEAD])
            for rb in sorted(set(rbs)):
                nc.sync.dma_start(out=gmono[rb:rb + KROWS, HEAD:],
                                  in_=gmono_d[:, HEAD:])
            featsw = constp.tile([cap, NPAIR * NCHP], bf16)
            nc.scalar.dma_start(out=featsw[:], in_=featsw_d[:])

            wgs = {}

            def sigma_pair(k):
                """Both sigma chunks of pair k in concurrent PE row groups
                (rows 64-75 / 96-107) into one 2-bank PSUM tile, one exp."""
                sg = pp.tile([cap, PIX2], f32, tag="sig", bufs=1,
                             name=f"sg{k}")
                for h in range(2):
                    gcol = PIX + (2 * k + h) * cap
                    rb = rbs[h]
                    nc.tensor.matmul(sg[:, h * PIX:(h + 1) * PIX],
                                     gmono[rb:rb + KROWS, gcol:gcol + cap],
                                     gmono[rb:rb + KROWS, 0:PIX],
                                     tile_position=(rb, 0),
                                     start=True, stop=True)
                wg = wp.tile([cap, PIX2], bf16, tag="w", name=f"wg{k}")
                nc.scalar.activation(wg[:], sg[:], EXP, scale=-1.0)
                wgs[k] = wg

            sigma_pair(0)
            sigma_pair(1)
            for k in range(NPAIR):               # pair loop
                if k % 2 == 0:
                    oas = op_.tile([128, 2 * PIX2], bf16, tag="oas")
                    obp = pp.tile([128, PIX2], f32, tag="obp", bufs=1)
                    obs = op_.tile([64, PIX2], bf16, tag="obs")
                wg = wgs.pop(k)
                oa = pp.tile([128, PIX2], f32, tag="oa", bufs=2,
                             name=f"oa{k}")
                cslot = 64 + 32 * (k % 2)
                for h in range(2):
                    px = slice(h * PIX, (h + 1) * PIX)
                    nc.tensor.matmul(oa[:, px],
                                     featsw[:, k * NCHP:k * NCHP + 128],
                                     wg[:, px], start=True, stop=True)
                    nc.tensor.matmul(
                        obp[cslot:cslot + 32, px],
                        featsw[:, k * NCHP + 128:(k + 1) * NCHP],
                        wg[:, px], tile_position=(0, cslot),
                        start=True, stop=True)
                if k + 2 < NPAIR:                # PE runs sigma 2 pairs ahead
                    sigma_pair(k + 2)
                nc.vector.tensor_copy(oas[:, (k % 2) * PIX2:(k % 2 + 1) * PIX2],
                                      oa[:])
                nc.sync.dma_start(
                    out=out_d[0:128, k * PIX2:(k + 1) * PIX2],
                    in_=oas[:, (k % 2) * PIX2:(k % 2 + 1) * PIX2])
                if k % 2 == 1:
                    nc.vector.tensor_copy(obs[:], obp[64:128, :])
                    g2 = k // 2
                    for kk in range(2):          # the 2 pairs in this group
                        kp = g2 * 2 + kk
                        nc.sync.dma_start(
                            out=out_d[128:NCH, kp * PIX2:(kp + 1) * PIX2],
                            in_=obs[32 * kk:32 * kk + 22, :])
    nc.compile()
    return nc


def _host_precompute(xyz_raw, cholesky_raw, opacity, features_dc, cluster_id):
    """Returns (cap, pmono, per-core gmat list, per-core featsw list)."""
    xyz = np.asarray(xyz_raw, np.float64)
    chol = np.asarray(cholesky_raw, np.float64)
    feats = np.asarray(features_dc, np.float64)[int(cluster_id)]  # [M, N, 3]

    xy = np.tanh(xyz)
    c = chol + np.array([0.5, 0.0, 0.5])
    l1, l2, l3 = c[:, 0], c[:, 1], c[:, 2]
    a = l1 * l1
    b = l1 * l2
    cc = l2 * l2 + l3 * l3
    det = a * cc - b * b
    c1, c2, c3 = cc / det, -b / det, a / det
    cx = 0.5 * ((xy[:, 0] + 1.0) * W - 1.0)
    cy = 0.5 * ((xy[:, 1] + 1.0) * H - 1.0)
    # opacity folds into the constant coefficient:
    # alpha = op*exp(-sigma) = exp(-(sigma - ln(op)))
    op = np.asarray(opacity, np.float64)[:, 0]
    ry = np.sqrt(np.maximum(2.0 * (LOG255 + np.log(np.maximum(op, 1e-30))),
                            0.0) * cc)

    # exact relevant-gaussian sets per 4-row pair (margin 0.25 for float
    # safety); max count measured 56 <= 64
    n_pairs_all = H // (2 * CHUNK_ROWS)
    psets = []
    for k in range(n_pairs_all):
        r0 = k * 2 * CHUNK_ROWS
        r1 = r0 + 2 * CHUNK_ROWS - 1
        rel = (cy + ry >= r0 - 0.25) & (cy - ry <= r1 + 0.25)
        psets.append(np.nonzero(rel)[0])
    maxcnt = max(len(s) for s in psets)
    cap = 64
    while cap < maxcnt:
        cap += 32
    assert cap <= 128, f"pair gaussian count {maxcnt} exceeds matmul cap"

    # feats_r[n, m*3+ch] = feats[m, n, ch]
    feats_r = np.ascontiguousarray(
        feats.transpose(1, 0, 2).reshape(N_GAUSS, NCH)).astype(np.float32)

    # fp32r-exact local monomial rows (dx' in [-127.5, 127.5] half-integers,
    # dy' = +-0.5); dx'^2 split into an exact hi/lo fp32r row pair.
    jj = np.arange(PIX)
    dxl = (jj % W) - 127.5
    dyl = (jj // W) - 0.5
    dx2 = dxl * dxl
    dx2_hi = _to_f32r(dx2).astype(np.float64)
    dx2_lo = dx2 - dx2_hi
    pmono = np.stack([dx2_hi, dx2_hi, dx2_lo, dyl * dyl,
                      dxl * dyl, dxl * dyl, dxl, dxl, dyl, dyl,
                      np.ones(PIX), np.ones(PIX)])

    gmats = []
    featsws = []
    for core in range(NCORES):
        gm = np.zeros((KROWS, NCHUNK * cap), np.float64)
        fw = np.zeros((cap, NPAIR * NCHP), np.float32)
        for q in range(NPAIR):
            k = core * NPAIR + q
            g = psets[k]
            cnt = len(g)
            if cnt == 0:
                continue
            fw[:cnt, q * NCHP:q * NCHP + NCH] = feats_r[g]
            gx = cx[g] - 127.5
            for h in range(2):
                pch = 2 * q + h
                gy = cy[g] - ((core * NCHUNK + pch) * CHUNK_ROWS + 0.5)
                # sigma = D*dx'^2 + E*dy'^2 + F*dx'dy' + B*dx' + C*dy' + A
                col = slice(pch * cap, pch * cap + cnt)
                D = 0.5 * c1[g]
                E = 0.5 * c3[g]
                F = c2[g]
                B = -(c1[g] * gx + c2[g] * gy)
                C = -(c3[g] * gy + c2[g] * gx)
                A = (0.5 * c1[g] * gx * gx + 0.5 * c3[g] * gy * gy
                     + c2[g] * gx * gy - np.log(np.maximum(op[g], 1e-30)))
                Dh = _to_f32r(D)
                Fh = _to_f32r(F)
                Bh = _to_f32r(B)
                Ch = _to_f32r(C)
                Ah = _to_f32r(A)
                gm[0, col] = Dh                  # * dx2_hi
                gm[1, col] = D - Dh             # * dx2_hi
                gm[2, col] = D                  # * dx2_lo
                gm[3, col] = E                  # * dy'^2
                gm[4, col] = Fh                 # * dx'dy'
                gm[5, col] = F - Fh
                gm[6, col] = Bh                 # * dx'
                gm[7, col] = B - Bh
                gm[8, col] = Ch                 # * dy'
                gm[9, col] = C - Ch
                gm[10, col] = Ah                # * 1
                gm[11, col] = A - Ah
        gmats.append(_to_f32r(gm))
        featsws.append(fw.astype(ml_dtypes.bfloat16))
    return cap, _to_f32r(pmono), gmats, featsws


def _in_maps(xyz_raw, cholesky_raw, opacity, features_dc, cluster_id):
    cap, pmono, gmats, featsws = _host_precompute(
        xyz_raw, cholesky_raw, opacity, features_dc, cluster_id)
    in_maps = [{"gmono": np.concatenate([pmono, gmats[c]], axis=1),
                "featsw": featsws[c]}
               for c in range(NCORES)]
    return cap, in_maps


def _assemble(results):
    full = np.concatenate([np.asarray(r["out"], np.float32)
                           for r in results], axis=1)      # [150, H*W]
    return np.ascontiguousarray(full.reshape(M_COMP, 3, H, W))


def _get_nc(cap):
    if cap not in _cache:
        _cache[cap] = _build_nc(cap)
    return _cache[cap]


def kernel(xyz_raw, cholesky_raw, opacity, features_dc, cluster_id):
    cap, in_maps = _in_maps(xyz_raw, cholesky_raw, opacity, features_dc,
                            cluster_id)
    nc = _get_nc(cap)
    res = run_bass_kernel_spmd(nc, in_maps, list(range(NCORES)))
    return _assemble(res.results)


def kernel_traced(xyz_raw, cholesky_raw, opacity, features_dc, cluster_id,
                  **trace_kwargs):
    """For test.py: returns (output, BassKernelResults with profile)."""
    cap, in_maps = _in_maps(xyz_raw, cholesky_raw, opacity, features_dc,
                            cluster_id)
    nc = _get_nc(cap)
    res = run_bass_kernel_spmd(nc, in_maps, list(range(NCORES)), trace=True,
                               **trace_kwargs)
    return _assemble(res.results), res
